# revision 15
# baseline (speedup 1.0000x reference)
"""Trainium2 Bass kernel for CausalSelfAttention (GQA, RoPE, prefill).

Tensor-parallel over the 8 query groups: core g owns query heads
[4g, 4g+4) and kv head g.  Each core computes a partial output
(full-shape, fp16) that the host sums in fp32.

The two dense projections (qkv: x@W_attn slice, proj: y@W_proj slice)
run as 3-term error-compensated fp8 DoubleRow matmuls:
    x @ W  ~=  x_hi@W_hi + x_lo@W_hi + x_hi@W_lo
with x_hi = e4m3(x), x_lo = e4m3(x - x_hi) (same for W, pre-scaled by
32 on host so W's ~N(0, 1/4096) entries stay in e4m3 normal range).
DoubleRow packs two (128-contraction-plane, term) pairs per PE
instruction at 0.5 cycles per output column, so each term costs 1/4 of
an fp16 matmul and the compensated product runs at 0.75x fp16 time with
~0.2% error (vs ~4% for naive fp8).  Scale bookkeeping: q,k are
descaled by folding 1/32 into the host cos/sin tables; v stays 32x and
W_proj adds another 32x, so the host divides the summed output by 1024.

Attention stays fp16: scores KV-MAJOR (scoresT = kT.T @ qT, 6-deep PSUM
rotation), exp on ACT straight into the PV rhs layout, causal-diagonal
mask on GpSimd, softmax denominator via DVE block pre-reduce + one
ones-matmul (cross-partition reduce + broadcast) + DVE reciprocal.
emit_pv additionally splits y into fp8 hi/lo for the proj stage
(DVE mul, ACT quantize-copy, GpSimd residual-subtract).

Schedule skeleton (every engine stream is in-order, so EMISSION ORDER
is the schedule): batch-0 qkv runs flat 6-PSUM kp-sweeps; batch 1 runs
three 2-PSUM m-groups per n-chunk with batch-0 attention interleaved;
batch-0 proj chunks interleave into batch-1's attention as PE filler.
"""

import os
import numpy as np

FLAGS = set(os.environ.get(
    'KFLAGS',
    'half_dma,tail_split,tail_alt,altcopy,late_part,vt_split').split(','))

B, T, NE, NH, NQG, HS = 2, 1024, 4096, 32, 8, 128
QPK = NH // NQG          # 4 query heads per kv group
NT = B * T               # 2048 tokens
GW = (QPK + 2) * HS      # 768 qkv rows per group
GQ = QPK * HS            # 512 q cols per group
P = 128
NCORES = 8
KC = NE // P             # 32 contraction chunks for qkv proj
KP = KC // 2             # 16 DoubleRow plane-pairs
MC = GW // P             # 6 qkv feature chunks
TC8 = T // P             # 8 token chunks per batch
NNC = NT // 512          # 4 token n-chunks
SCALE = 1.0 / float(np.sqrt(HS))
WSCALE = 32.0            # host pre-scale on W_attn / W_proj before e4m3

_CACHE = {}


def _split_waits(nc, mybir, max_waits=1):
    """walrus in this container rejects >1 sync-wait per instruction;
    hoist extras onto single-wait NoOps just before (equivalent since
    semaphores are monotonic and a sequencer executes in order)."""
    for fn in nc.m.functions:
        for blk in fn.blocks:
            new_list, changed = [], False
            for inst in blk.instructions:
                si = getattr(inst, "sync_info", None)
                if si is not None and len(si.on_wait) > max_waits:
                    waits = list(si.on_wait)
                    for i, w in enumerate(waits[:-max_waits]):
                        nop = mybir.InstNoOp(
                            name=f"{inst.name}-wsplit-{i}", ins=[], outs=[],
                            engine=inst.engine)
                        nop.sync_info = mybir.SyncInfo(on_wait=[w], on_update=[])
                        new_list.append(nop)
                    inst.sync_info = mybir.SyncInfo(
                        on_wait=waits[-max_waits:], on_update=list(si.on_update))
                    changed = True
                new_list.append(inst)
            if changed:
                blk.instructions = new_list


def _build_nc(reps=1, split_waits=True):
    import concourse.bass as bass
    import concourse.mybir as mybir
    import concourse.tile as tile
    from contextlib import ExitStack

    f32 = mybir.dt.float32
    f16 = mybir.dt.float16
    f8 = mybir.dt.float8e4
    DR = mybir.MatmulPerfMode.DoubleRow

    nc = bass.Bass()
    # fp8 pair-packed inputs, hi/lo fused per DMA (see _host_prep)
    x8_d = nc.dram_tensor("x8", [NNC, KP, P, 2, 2, 512], f8,
                          kind="ExternalInput")
    wq8_d = nc.dram_tensor("wq8", [KP, P, 2, MC, 2, P], f8,
                           kind="ExternalInput")
    wp8_d = nc.dram_tensor("wp8", [2, P, 2, NE // 512, 2, 512], f8,
                           kind="ExternalInput")
    css_d = nc.dram_tensor("css", [P, 2, NT], f16, kind="ExternalInput")
    misc_d = nc.dram_tensor("misc", [P, 3, P], f16, kind="ExternalInput")
    out_d = nc.dram_tensor("out", [NT, NE], f16, kind="ExternalOutput")

    # column offset of kv-chunk c's block inside the expT tile
    offs, acc = [], 0
    for c in range(TC8):
        offs.append(acc)
        acc += (TC8 - c) * P

    with tile.TileContext(nc) as tc:
      for _rep in range(reps):
        sL = ExitStack()   # left-side long-lived pools (y8, wp8, ob)
        sR = ExitStack()   # right-side pools (qk16, attention-era)
        try:
            # const: left
            const = sL.enter_context(tc.tile_pool(name="const", bufs=1))
            css = const.tile([P, 2, NT], f16)
            cc, ss = css[:, 0], css[:, 1]
            misc = const.tile([P, 3, P], f16)
            maskT, ones2d, ident16 = misc[:, 0], misc[:, 1], misc[:, 2]

            def alloc_yw_pools():
                y_pool = sL.enter_context(tc.tile_pool(name="y", bufs=1))
                # (p, kp-pair, token-chunk, slot, col) — proj lhsT slices
                y8h = y_pool.tile([P, 2, NT // P, 2, P], f8)
                y8l = y_pool.tile([P, 2, NT // P, 2, P], f8)
                wp_pool = sL.enter_context(tc.tile_pool(name="wp", bufs=1))
                wp8 = wp_pool.tile([P, 2, 2, NE // 512, 2, 512], f8)
                return y8h, y8l, wp8

            # qk16 on the right: lives through attention
            qk16 = sR.enter_context(tc.tile_pool(name="qk16", bufs=1, side="right"))
            q16 = qk16.tile([P, QPK, NT], f16)
            k16 = qk16.tile([P, NT], f16)
            vtm = qk16.tile([P, B * TC8, P], f16)

            att = {}

            def make_att_pools():
                att['expT'] = sR.enter_context(
                    tc.tile_pool(name="expT", bufs=5, side="right"))
                att['part'] = sR.enter_context(
                    tc.tile_pool(name="part", bufs=2, side="right"))
                att['rb'] = sR.enter_context(
                    tc.tile_pool(name="rb", bufs=2, side="right"))
                att['y16'] = sR.enter_context(
                    tc.tile_pool(name="y16", bufs=2, side="right"))
                att['psA'] = sR.enter_context(
                    tc.tile_pool(name="psA", bufs=1, space="PSUM"))

            expTs = {}
            parts = {}
            rbs = {}

            def emit_partial(b, hc):
                """DVE pre-reduce of the kv-chunk blocks into partial[128,T]
                (softmax denominator before the cross-partition reduce).
                Emitted only once exp data is near-ready so the in-order DVE
                queue is not head-of-line blocked."""
                expT = expTs[(b, hc)]
                eng = nc.gpsimd if b == 0 else nc.vector
                part = att['part'].tile([P, T], f16, tag="part",
                                        name=f"part{b}_{hc}")
                parts[(b, hc)] = part
                eng.tensor_copy(part[:], expT[:, offs[0]:offs[0] + T])
                with nc.allow_low_precision(
                        reason="fp16 partial rowsums; d<=~3e3, tol 2e-2"):
                    for c in range(1, TC8):
                        w = T - c * P
                        eng.tensor_add(
                            part[:, c * P:T], part[:, c * P:T],
                            expT[:, offs[c]:offs[c] + w])

            def emit_dps(b, hc):
                """softmax denominator: ones-matmul does the cross-partition
                reduce AND the 128-way broadcast; DVE reciprocal to SBUF."""
                psA = att['psA']
                part = parts.pop((b, hc))
                rb = att['rb'].tile([P, T], f16, tag="rb", name=f"rb{b}_{hc}")
                rbs[(b, hc)] = rb
                for (q0, q1) in ((0, 512), (512, T)):
                    dps = psA.tile([P, 512], f32, tag="acc", bufs=6,
                                   name=f"dps{b}_{hc}_{q0}")
                    nc.tensor.matmul(dps[:], ones2d, part[:, q0:q1],
                                     start=True, stop=True)
                    with nc.allow_low_precision(
                            reason="fp16 1/d; d in [1,~3e3], tol 2e-2"):
                        nc.vector.reciprocal(rb[:, q0:q1], dps[:])

            def emit_scores(b, hc):
                """scoresT = kT.T @ qT per kv chunk, exp on ACT, causal mask
                on GpSimd."""
                psA = att['psA']
                tok = slice(b * T, (b + 1) * T)
                qT_i = q16[:, hc, tok]
                expT = att['expT'].tile([P, acc], f16, tag="expT",
                                        name=f"expT{b}_{hc}")
                expTs[(b, hc)] = expT
                for c in range(TC8):
                    kT_c = k16[:, b * T + c * P: b * T + (c + 1) * P]
                    spans = [(c * P, 512)] if c < 4 else []
                    spans += [(max(512, c * P), T)]
                    for si, (q0, q1) in enumerate(spans):
                        sps = psA.tile([P, 512], f32, tag="acc", bufs=6,
                                       name=f"sps{b}_{hc}_{c}_{q0}")
                        w = q1 - q0
                        nc.tensor.matmul(sps[:, :w], kT_c,
                                         qT_i[:, q0:q1],
                                         start=True, stop=True)
                        eo = offs[c] + (q0 - c * P)
                        nc.scalar.activation(
                            expT[:, eo:eo + w], sps[:, :w],
                            mybir.ActivationFunctionType.Exp, scale=SCALE)
                    # zero the invalid (kv > q) half of the diagonal block
                    nc.gpsimd.tensor_mul(
                        expT[:, offs[c]:offs[c] + P],
                        expT[:, offs[c]:offs[c] + P], maskT)
                if not (b == 1 and 'late_part' in FLAGS):
                    emit_partial(b, hc)

            def emit_pv(b, hc):
                """y = probs @ v (unnormalized), denominator reduce+broadcast
                via ones-matmul, DVE reciprocal, normalizing multiply into a
                fp16 scratch, then fp8 hi/lo split for the proj stage."""
                psA = att['psA']
                if (b, hc) not in parts and (b, hc) not in rbs:
                    emit_partial(b, hc)
                expT = expTs.pop((b, hc))
                yps = att['psY'].tile([P, T], f32, tag="yps", bufs=1,
                                      name=f"yps{b}_{hc}")
                for (s0, s1) in ((0, 512), (512, T)):
                    cs = [c for c in range(TC8) if c * P < s1]
                    for c in cs:
                        q0 = max(s0, c * P)
                        sl = slice(offs[c] + (q0 - c * P),
                                   offs[c] + (s1 - c * P))
                        nc.tensor.matmul(
                            yps[:, q0:s1], vtm[:, b * TC8 + c, :],
                            expT[:, sl], start=(c == 0), stop=(c == cs[-1]))
                if (b, hc) not in rbs:
                    emit_dps(b, hc)
                rb = rbs.pop((b, hc))
                y16 = att['y16'].tile([P, T], f16, tag="y16",
                                      name=f"y16_{b}_{hc}")
                nc.vector.tensor_mul(y16[:], yps[:], rb[:])
                kp, sl8 = hc // 2, hc % 2
                yh_v = y8h[:, kp, b * TC8:(b + 1) * TC8, sl8, :]
                yl_v = y8l[:, kp, b * TC8:(b + 1) * TC8, sl8, :]
                nc.scalar.copy(yh_v, y16[:])
                with nc.allow_low_precision(
                        reason="fp8 hi/lo split; recon err ~0.1%, tol 2e-2"):
                    nc.gpsimd.tensor_sub(yl_v, y16[:], yh_v)

            def emit_proj(m, era='tail'):
                """out[tokens m*128:(m+1)*128, :] = y.T @ wproj (partial),
                3-term compensated fp8 DoubleRow."""
                psA = att['psA']
                def _cp_scalar(dst, srcv):
                    nc.scalar.copy(dst, srcv)

                def _cp_vector(dst, srcv):
                    nc.vector.tensor_copy(dst, srcv)

                engs = ((_cp_scalar, _cp_vector) if era == 'plan'
                        else (_cp_scalar, _cp_vector))
                ob = ob_pool.tile([P, NE], f16, tag="ob", name=f"ob{m}")
                for n in range(NE // 512):
                    opsum = psA.tile([P, 512], f32, tag="acc", bufs=6,
                                     name=f"ops{m}_{n}")
                    for kp in range(2):
                        for t, (yt, wv) in enumerate(
                                ((y8h, 0), (y8l, 0), (y8h, 1))):
                            nc.tensor.matmul(
                                opsum[:], yt[:, kp, m], wp8[:, kp, wv, n],
                                start=(kp == 0 and t == 0),
                                stop=(kp == 1 and t == 2), perf_mode=DR)
                    if 'tail_alt' in FLAGS and m == NT // P - 1:
                        if n % 2:
                            nc.vector.tensor_copy(
                                ob[:, n * 512:(n + 1) * 512], opsum[:])
                        else:
                            nc.scalar.copy(
                                ob[:, n * 512:(n + 1) * 512], opsum[:])
                    else:
                        engs[n % len(engs)](
                            ob[:, n * 512:(n + 1) * 512], opsum[:])
                    if 'tail_split' in FLAGS and m == NT // P - 1:
                        c0, c1 = n * 512, (n + 1) * 512
                        nc.sync.dma_start(
                            out_d[m * P:(m + 1) * P, c0:c1], ob[:, c0:c1])
                    elif 'quarter_dma' in FLAGS:
                        if n % 2 == 1:
                            c0, c1 = (n - 1) * 512, (n + 1) * 512
                            nc.sync.dma_start(
                                out_d[m * P:(m + 1) * P, c0:c1], ob[:, c0:c1])
                    elif 'half_dma' in FLAGS and n % 4 == 3:
                        c0, c1 = (n - 3) * 512, (n + 1) * 512
                        nc.sync.dma_start(
                            out_d[m * P:(m + 1) * P, c0:c1], ob[:, c0:c1])
                    elif n == 3:
                        nc.sync.dma_start(
                            out_d[m * P:(m + 1) * P, 0:2048], ob[:, 0:2048])
                    elif n == 7:
                        nc.sync.dma_start(
                            out_d[m * P:(m + 1) * P, 2048:NE], ob[:, 2048:NE])

            # ============ phase 1+2: qkv projection + rope, per batch ========
            with ExitStack() as sA:
                qkv_pool = sA.enter_context(tc.tile_pool(name="qkv", bufs=1))
                qkv = qkv_pool.tile([P, MC, NT], f16)
                wq_pool = sA.enter_context(tc.tile_pool(name="wq", bufs=1))
                wq8 = wq_pool.tile([P, KP, 2, MC, 2, P], f8)
                xs_pool = sA.enter_context(tc.tile_pool(name="xs", bufs=4))
                rp = sA.enter_context(tc.tile_pool(name="rope", bufs=2))

                def rope_span(b, tok, w):
                    h = HS // 2
                    ccb, ssb = cc[:, tok], ss[:, tok]
                    # half-spans get their own tags: mixed tile sizes inside
                    # one rotation tag alias SBUF and corrupt data on HW
                    sfx = "" if w == T else "H"
                    for hc in [QPK] + list(range(QPK)):
                        src_ = qkv[:, hc, tok]
                        rot = rp.tile([P, w], f16, tag="rot" + sfx,
                                      name=f"rot{b}_{hc}_{tok.start}")
                        nc.sync.dma_start(rot[0:h, :], src_[h:P, :])
                        nc.sync.dma_start(rot[h:P, :], src_[0:h, :])
                        t1 = rp.tile([P, w], f16, tag="t1" + sfx,
                                     name=f"t1_{b}_{hc}_{tok.start}")
                        t2 = rp.tile([P, w], f16, tag="t2" + sfx,
                                     name=f"t2_{b}_{hc}_{tok.start}")
                        nc.vector.tensor_mul(t1[:], src_, ccb)
                        nc.vector.tensor_mul(t2[:], rot[:], ssb)
                        dst = (q16[:, hc, tok] if hc < QPK
                               else k16[:, tok])
                        with nc.allow_low_precision(
                                reason="fp16 rope; |q|,|k|~1, tol 2e-2"):
                            nc.vector.tensor_add(dst, t1[:], t2[:])

                def rope_batch(b):
                    rope_span(b, slice(b * T, (b + 1) * T), T)

                def vt_batch(b, pool, tag, bufs, shape, cs=None):
                    for c in (range(TC8) if cs is None else cs):
                        # PE transpose (avoids XBAR DMA-transpose, which
                        # races concurrent DMA copies on this stack)
                        vt_ps = pool.tile(shape, f16, tag=tag, bufs=bufs,
                                          name=f"vt{b}_{c}")
                        nc.tensor.transpose(
                            vt_ps[:, 0:P],
                            qkv[:, QPK + 1, b * T + c * P: b * T + (c + 1) * P],
                            ident16)
                        nc.any.tensor_copy(
                            vtm[:, b * TC8 + c, :], vt_ps[:, 0:P])

                def consts_dma():
                    # one fused DMA for cos/sin tables, one for mask/ones/ident
                    nc.sync.dma_start(css[:], css_d[:])
                    nc.sync.dma_start(misc[:], misc_d[:])

                # resident x8 hi/lo tile sets shared by both batches;
                # hi+lo fused into one DMA to halve HWDGE desc-gen slots
                def load_xt(n, kp):
                    xt = xs_pool.tile([P, 2, 2, 512], f8, tag="xt",
                                      bufs=16, name=f"xt{n}_{kp}")
                    nc.sync.dma_start(xt[:], x8_d[n, kp])
                    return xt[:, 0], xt[:, 1]

                def qkv_mms(psums, ms, kp, xh, xl, first, last,
                            term_major=False):
                    """3-term compensated fp8 DR matmuls for one kp pair."""
                    terms = ((0, xh), (1, xh), (0, xl))
                    if term_major:
                        for t, (wv, xt) in enumerate(terms):
                            for m in ms:
                                nc.tensor.matmul(
                                    psums[m][:], wq8[:, kp, wv, m], xt,
                                    start=(first and t == 0),
                                    stop=(last and t == 2), perf_mode=DR)
                    else:
                        for m in ms:
                            for t, (wv, xt) in enumerate(terms):
                                nc.tensor.matmul(
                                    psums[m][:], wq8[:, kp, wv, m], xt,
                                    start=(first and t == 0),
                                    stop=(last and t == 2), perf_mode=DR)

                # ---- batch 0: flat 6-psum sweeps, own PSUM pool ----
                with ExitStack() as sP0:
                    ps1a = sP0.enter_context(
                        tc.tile_pool(name="ps1a", bufs=7, space="PSUM"))
                    for n in (0, 1):
                        psums = [ps1a.tile([P, 512], f32, tag="ps1",
                                           name=f"ps1_{n}_{m_}")
                                 for m_ in range(MC)]
                        for kp in range(KP):
                            if n == 0 and kp == 0:
                                # split the very first loads so PE starts on
                                # the hi-term sweep while lo is in flight
                                nc.sync.dma_start(wq8[:, 0, 0], wq8_d[0, :, 0])
                                xh, xl = load_xt(n, kp)
                                nc.sync.dma_start(wq8[:, 0, 1], wq8_d[0, :, 1])
                                qkv_mms(psums, range(MC), kp, xh, xl,
                                        True, False, term_major=True)
                                continue
                            if n == 0:
                                nc.sync.dma_start(wq8[:, kp], wq8_d[kp])
                            xh, xl = load_xt(n, kp)
                            qkv_mms(psums, range(MC), kp, xh, xl,
                                    kp == 0, kp == KP - 1)
                        for m in range(MC):
                            if 'altcopy' in FLAGS and m % 2 == 0:
                                nc.scalar.copy(
                                    qkv[:, m, n * 512:(n + 1) * 512],
                                    psums[m][:])
                            else:
                                nc.vector.tensor_copy(
                                    qkv[:, m, n * 512:(n + 1) * 512],
                                    psums[m][:])
                        if n == 0:
                            # MUST precede the first vt transpose: a
                            # reader emitted before its producer DMA
                            # gets no dependency and reads uninitialized
                            # SBUF (ident16 is vt's identity operand)
                            consts_dma()
                        if 'vt_split' in FLAGS:
                            vt_batch(0, ps1a, "vt", 1, [P, P],
                                     cs=range(n * 4, n * 4 + 4))
                    rope_batch(0)
                    if 'vt_split' not in FLAGS:
                        vt_batch(0, ps1a, "vt", 1, [P, P])

                # attention pools come alive before batch 1 so batch-0
                # scores/exp interleave into batch-1's qkv stream
                make_att_pools()

                # ---- batch 1: three 2-psum m-groups per n-chunk ----
                groups = ((0, 1), (2, 3), (4, 5))
                with ExitStack() as sP1:
                    ps1b = sP1.enter_context(
                        tc.tile_pool(name="ps1b", bufs=2, space="PSUM"))
                    for n in (2, 3):
                        xts = {}
                        for g, ms in enumerate(groups):
                            psums = {m_: ps1b.tile(
                                [P, 512], f32, tag="ps1",
                                name=f"ps1_{n}_{g}_{m_}") for m_ in ms}
                            for kp in range(KP):
                                if g == 0:
                                    xts[kp] = load_xt(n, kp)
                                qkv_mms(psums, ms, kp, *xts[kp],
                                        kp == 0, kp == KP - 1)
                            for m in ms:
                                nc.vector.tensor_copy(
                                    qkv[:, m, n * 512:(n + 1) * 512],
                                    psums[m][:])
                            slot = (n - 2) * 3 + g
                            if slot < QPK:
                                emit_scores(0, slot)
                    rope_batch(1)
                    vt_batch(1, att['psA'], "acc", 6, [P, 1024])

            # yps PSUM pool only comes alive after ps1b frees its banks
            att['psY'] = sR.enter_context(
                tc.tile_pool(name="psY", bufs=1, space="PSUM"))
            y8h, y8l, wp8 = alloc_yw_pools()
            ob_pool = sL.enter_context(tc.tile_pool(name="ob", bufs=2))
            for kp in range(2):
                nc.sync.dma_start(wp8[:, kp], wp8_d[kp])

            # ===== batch 0 attention (pv) / batch 1 scores interleave =====
            for i in range(QPK):
                emit_pv(0, i)
                emit_scores(1, i)

            # ===== batch 1 attention interleaved with batch-0 proj: proj
            # matmuls keep PE busy while ACT runs exp for the next head =====
            plan = [('pt', 0), ('pj', 0), ('pt', 1), ('pv', 0),
                    ('pj', 1), ('pt', 2), ('pj', 2), ('pv', 1),
                    ('pj', 3), ('pt', 3), ('pj', 4), ('pv', 2),
                    ('pj', 5), ('pj', 6), ('pv', 3), ('pj', 7)]
            for op, i in plan:
                if op == 'pt':
                    if 'late_part' in FLAGS:
                        emit_partial(1, i)
                elif op == 'pv':
                    emit_pv(1, i)
                else:
                    emit_proj(i, era='plan')
            for m in range(8, NT // P):
                emit_proj(m)
        finally:
            sR.close()
            sL.close()

    if split_waits:
        _split_waits(nc, mybir)
    return nc


def _q8(v):
    import ml_dtypes
    return np.ascontiguousarray(v).astype(ml_dtypes.float8_e4m3)


def _split8(v):
    """2-level e4m3 decomposition: v ~= hi + lo."""
    hi = _q8(v)
    lo = _q8(v - hi.astype(np.float32))
    return hi, lo


def _host_prep(x, cos, sin, W_attn, W_proj):
    xT = np.ascontiguousarray(x.reshape(NT, NE).T)          # [NE, NT] f32
    # x8[n, kp, p, which, i, c] = e4m3{,resid}(xT[(2kp+i)*P + p, n*512 + c])
    xr = xT.reshape(KP, 2, P, NNC, 512).transpose(3, 0, 2, 1, 4)
    x8h, x8l = _split8(xr)
    x8 = np.ascontiguousarray(np.stack([x8h, x8l], axis=3))
    cosT = np.tile(cos.T, (1, B)) / WSCALE
    sinT = np.tile(sin.T, (1, B)) / WSCALE
    cc = np.concatenate([cosT, cosT], axis=0)
    ss = np.concatenate([-sinT, sinT], axis=0)
    css = np.ascontiguousarray(
        np.stack([cc, ss], axis=1), dtype=np.float16)
    # scoresT layout [kv, q]: zero strictly-lower (kv > q) entries post-exp
    maskT = np.triu(np.ones((P, P), dtype=np.float32))
    misc = np.ascontiguousarray(np.stack(
        [maskT, np.ones((P, P), dtype=np.float32), np.eye(P)],
        axis=1), dtype=np.float16)
    common = {"x8": x8, "css": css, "misc": misc}
    in_maps = []
    for g in range(NCORES):
        m = dict(common)
        wq = W_attn[g * GW:(g + 1) * GW, :].T * WSCALE      # [NE, GW] f32
        # wq8[kp, p, which, m, i, j] = e4m3{,resid}(32*wq[(2kp+i)*P+p, m*128+j])
        wqr = wq.reshape(KP, 2, P, MC, P).transpose(0, 2, 3, 1, 4)
        m["wq8"] = np.ascontiguousarray(np.stack(_split8(wqr), axis=2))
        wp = W_proj[:, g * GQ:(g + 1) * GQ].T * WSCALE      # [GQ, NE] f32
        # wp8[kp, p, which, nn, i, c] = e4m3{,resid}(32*wp[(2kp+i)*P+p, nn*512+c])
        wpr = wp.reshape(2, 2, P, NE // 512, 512).transpose(0, 2, 3, 1, 4)
        m["wp8"] = np.ascontiguousarray(np.stack(_split8(wpr), axis=2))
        in_maps.append(m)
    return in_maps


LAST_EXEC_NS = None


def kernel(x, cos, sin, W_attn, W_proj, max_seq_length):
    global LAST_EXEC_NS
    from concourse.bass_utils import run_bass_kernel_spmd

    x = np.asarray(x, dtype=np.float32)
    cos = np.asarray(cos, dtype=np.float32)
    sin = np.asarray(sin, dtype=np.float32)
    W_attn = np.asarray(W_attn, dtype=np.float32)
    W_proj = np.asarray(W_proj, dtype=np.float32)

    if "nc" not in _CACHE:
        _CACHE["nc"] = _build_nc()
    nc = _CACHE["nc"]

    in_maps = _host_prep(x, cos, sin, W_attn, W_proj)
    res = run_bass_kernel_spmd(nc, in_maps, core_ids=list(range(NCORES)))
    LAST_EXEC_NS = res.exec_time_ns

    acc = res.results[0]["out"].astype(np.float32)
    for g in range(1, NCORES):
        acc = acc + res.results[g]["out"].astype(np.float32)
    return acc.reshape(B, T, NE) * (1.0 / (WSCALE * WSCALE))


# revision 16
# speedup vs baseline: 1.0214x; 1.0214x over previous
"""Trainium2 Bass kernel for CausalSelfAttention (GQA, RoPE, prefill).

Tensor-parallel over the 8 query groups: core g owns query heads
[4g, 4g+4) and kv head g.  Each core computes a partial output
(full-shape, fp16) that the host sums in fp32.

The two dense projections (qkv: x@W_attn slice, proj: y@W_proj slice)
run as 3-term error-compensated fp8 DoubleRow matmuls:
    x @ W  ~=  x_hi@W_hi + x_lo@W_hi + x_hi@W_lo
with x_hi = e4m3(x), x_lo = e4m3(x - x_hi) (same for W, pre-scaled by
32 on host so W's ~N(0, 1/4096) entries stay in e4m3 normal range).
DoubleRow packs two (128-contraction-plane, term) pairs per PE
instruction at 0.5 cycles per output column, so each term costs 1/4 of
an fp16 matmul and the compensated product runs at 0.75x fp16 time with
~0.2% error (vs ~4% for naive fp8).  Scale bookkeeping: q,k are
descaled by folding 1/32 into the host cos/sin tables; v stays 32x and
W_proj adds another 32x, so the host divides the summed output by 1024.

Attention stays fp16: scores KV-MAJOR (scoresT = kT.T @ qT, 6-deep PSUM
rotation), exp on ACT straight into the PV rhs layout, causal-diagonal
mask on GpSimd, softmax denominator via DVE block pre-reduce + one
ones-matmul (cross-partition reduce + broadcast) + DVE reciprocal.
emit_pv additionally splits y into fp8 hi/lo for the proj stage
(DVE mul, ACT quantize-copy, GpSimd residual-subtract).

Schedule skeleton (every engine stream is in-order, so EMISSION ORDER
is the schedule): batch-0 qkv runs flat 6-PSUM kp-sweeps; batch 1 runs
three 2-PSUM m-groups per n-chunk with batch-0 attention interleaved;
batch-0 proj chunks interleave into batch-1's attention as PE filler.
"""

import os
import numpy as np

FLAGS = set(os.environ.get(
    'KFLAGS',
    'half_dma,tail_split,tail_alt,altcopy,late_part,vt_split').split(','))

B, T, NE, NH, NQG, HS = 2, 1024, 4096, 32, 8, 128
QPK = NH // NQG          # 4 query heads per kv group
NT = B * T               # 2048 tokens
GW = (QPK + 2) * HS      # 768 qkv rows per group
GQ = QPK * HS            # 512 q cols per group
P = 128
NCORES = 8
KC = NE // P             # 32 contraction chunks for qkv proj
KP = KC // 2             # 16 DoubleRow plane-pairs
MC = GW // P             # 6 qkv feature chunks
TC8 = T // P             # 8 token chunks per batch
NNC = NT // 512          # 4 token n-chunks
SCALE = 1.0 / float(np.sqrt(HS))
WSCALE = 32.0            # host pre-scale on W_attn / W_proj before e4m3

_CACHE = {}


def _split_waits(nc, mybir, max_waits=1):
    """walrus in this container rejects >1 sync-wait per instruction;
    hoist extras onto single-wait NoOps just before (equivalent since
    semaphores are monotonic and a sequencer executes in order)."""
    for fn in nc.m.functions:
        for blk in fn.blocks:
            new_list, changed = [], False
            for inst in blk.instructions:
                si = getattr(inst, "sync_info", None)
                if si is not None and len(si.on_wait) > max_waits:
                    waits = list(si.on_wait)
                    for i, w in enumerate(waits[:-max_waits]):
                        nop = mybir.InstNoOp(
                            name=f"{inst.name}-wsplit-{i}", ins=[], outs=[],
                            engine=inst.engine)
                        nop.sync_info = mybir.SyncInfo(on_wait=[w], on_update=[])
                        new_list.append(nop)
                    inst.sync_info = mybir.SyncInfo(
                        on_wait=waits[-max_waits:], on_update=list(si.on_update))
                    changed = True
                new_list.append(inst)
            if changed:
                blk.instructions = new_list


def _build_nc(reps=1, split_waits=True):
    import concourse.bass as bass
    import concourse.mybir as mybir
    import concourse.tile as tile
    from contextlib import ExitStack

    f32 = mybir.dt.float32
    f16 = mybir.dt.float16
    f8 = mybir.dt.float8e4
    DR = mybir.MatmulPerfMode.DoubleRow

    nc = bass.Bass()
    # fp8 pair-packed inputs, hi/lo fused per DMA (see _host_prep)
    x8_d = nc.dram_tensor("x8", [NNC, KP, P, 2, 2, 512], f8,
                          kind="ExternalInput")
    wq8_d = nc.dram_tensor("wq8", [KP, P, 2, MC, 2, P], f8,
                           kind="ExternalInput")
    wp8_d = nc.dram_tensor("wp8", [2, P, 2, NE // 512, 2, 512], f8,
                           kind="ExternalInput")
    css_d = nc.dram_tensor("css", [P, 2, NT], f16, kind="ExternalInput")
    misc_d = nc.dram_tensor("misc", [P, 3, P], f16, kind="ExternalInput")
    out_d = nc.dram_tensor("out", [NT, NE], f16, kind="ExternalOutput")

    # column offset of kv-chunk c's block inside the expT tile
    offs, acc = [], 0
    for c in range(TC8):
        offs.append(acc)
        acc += (TC8 - c) * P

    with tile.TileContext(nc) as tc:
      for _rep in range(reps):
        sL = ExitStack()   # left-side long-lived pools (y8, wp8, ob)
        sR = ExitStack()   # right-side pools (qk16, attention-era)
        try:
            # const: left
            const = sL.enter_context(tc.tile_pool(name="const", bufs=1))
            css = const.tile([P, 2, NT], f16)
            cc, ss = css[:, 0], css[:, 1]
            misc = const.tile([P, 3, P], f16)
            maskT, ones2d, ident16 = misc[:, 0], misc[:, 1], misc[:, 2]

            def alloc_yw_pools():
                y_pool = sL.enter_context(tc.tile_pool(name="y", bufs=1))
                # (p, kp-pair, token-chunk, slot, col) — proj lhsT slices
                y8h = y_pool.tile([P, 2, NT // P, 2, P], f8)
                y8l = y_pool.tile([P, 2, NT // P, 2, P], f8)
                wp_pool = sL.enter_context(tc.tile_pool(name="wp", bufs=1))
                wp8 = wp_pool.tile([P, 2, 2, NE // 512, 2, 512], f8)
                return y8h, y8l, wp8

            # qk16 on the right: lives through attention
            qk16 = sR.enter_context(tc.tile_pool(name="qk16", bufs=1, side="right"))
            q16 = qk16.tile([P, QPK, NT], f16)
            k16 = qk16.tile([P, NT], f16)
            vtm = qk16.tile([P, B * TC8, P], f16)

            att = {}

            def make_att_pools():
                att['expT'] = sR.enter_context(
                    tc.tile_pool(name="expT", bufs=5, side="right"))
                att['part'] = sR.enter_context(
                    tc.tile_pool(name="part", bufs=2, side="right"))
                att['rb'] = sR.enter_context(
                    tc.tile_pool(name="rb", bufs=2, side="right"))
                att['y16'] = sR.enter_context(
                    tc.tile_pool(name="y16", bufs=2, side="right"))
                att['psA'] = sR.enter_context(
                    tc.tile_pool(name="psA", bufs=1, space="PSUM"))

            expTs = {}
            parts = {}
            rbs = {}

            def emit_partial(b, hc):
                """DVE pre-reduce of the kv-chunk blocks into partial[128,T]
                (softmax denominator before the cross-partition reduce).
                Emitted only once exp data is near-ready so the in-order DVE
                queue is not head-of-line blocked."""
                expT = expTs[(b, hc)]
                eng = nc.vector
                part = att['part'].tile([P, T], f16, tag="part",
                                        name=f"part{b}_{hc}")
                parts[(b, hc)] = part
                eng.tensor_copy(part[:], expT[:, offs[0]:offs[0] + T])
                with nc.allow_low_precision(
                        reason="fp16 partial rowsums; d<=~3e3, tol 2e-2"):
                    for c in range(1, TC8):
                        w = T - c * P
                        eng.tensor_add(
                            part[:, c * P:T], part[:, c * P:T],
                            expT[:, offs[c]:offs[c] + w])

            def emit_dps(b, hc):
                """softmax denominator: ones-matmul does the cross-partition
                reduce AND the 128-way broadcast; DVE reciprocal to SBUF."""
                psA = att['psA']
                part = parts.pop((b, hc))
                rb = att['rb'].tile([P, T], f16, tag="rb", name=f"rb{b}_{hc}")
                rbs[(b, hc)] = rb
                for (q0, q1) in ((0, 512), (512, T)):
                    dps = psA.tile([P, 512], f32, tag="acc", bufs=6,
                                   name=f"dps{b}_{hc}_{q0}")
                    nc.tensor.matmul(dps[:], ones2d, part[:, q0:q1],
                                     start=True, stop=True)
                    with nc.allow_low_precision(
                            reason="fp16 1/d; d in [1,~3e3], tol 2e-2"):
                        nc.vector.reciprocal(rb[:, q0:q1], dps[:])

            def emit_scores(b, hc):
                """scoresT = kT.T @ qT per kv chunk, exp on ACT, causal mask
                on GpSimd."""
                psA = att['psA']
                tok = slice(b * T, (b + 1) * T)
                qT_i = q16[:, hc, tok]
                expT = att['expT'].tile([P, acc], f16, tag="expT",
                                        name=f"expT{b}_{hc}")
                expTs[(b, hc)] = expT
                for c in range(TC8):
                    kT_c = k16[:, b * T + c * P: b * T + (c + 1) * P]
                    spans = [(c * P, 512)] if c < 4 else []
                    spans += [(max(512, c * P), T)]
                    for si, (q0, q1) in enumerate(spans):
                        sps = psA.tile([P, 512], f32, tag="acc", bufs=6,
                                       name=f"sps{b}_{hc}_{c}_{q0}")
                        w = q1 - q0
                        nc.tensor.matmul(sps[:, :w], kT_c,
                                         qT_i[:, q0:q1],
                                         start=True, stop=True)
                        eo = offs[c] + (q0 - c * P)
                        nc.scalar.activation(
                            expT[:, eo:eo + w], sps[:, :w],
                            mybir.ActivationFunctionType.Exp, scale=SCALE)
                    # zero the invalid (kv > q) half of the diagonal block
                    nc.gpsimd.tensor_mul(
                        expT[:, offs[c]:offs[c] + P],
                        expT[:, offs[c]:offs[c] + P], maskT)
                if not (b == 1 and 'late_part' in FLAGS):
                    emit_partial(b, hc)

            def emit_pv(b, hc):
                """y = probs @ v (unnormalized), denominator reduce+broadcast
                via ones-matmul, DVE reciprocal, normalizing multiply into a
                fp16 scratch, then fp8 hi/lo split for the proj stage."""
                psA = att['psA']
                if (b, hc) not in parts and (b, hc) not in rbs:
                    emit_partial(b, hc)
                expT = expTs.pop((b, hc))
                yps = att['psY'].tile([P, T], f32, tag="yps", bufs=1,
                                      name=f"yps{b}_{hc}")
                for (s0, s1) in ((0, 512), (512, T)):
                    cs = [c for c in range(TC8) if c * P < s1]
                    for c in cs:
                        q0 = max(s0, c * P)
                        sl = slice(offs[c] + (q0 - c * P),
                                   offs[c] + (s1 - c * P))
                        nc.tensor.matmul(
                            yps[:, q0:s1], vtm[:, b * TC8 + c, :],
                            expT[:, sl], start=(c == 0), stop=(c == cs[-1]))
                if (b, hc) not in rbs:
                    emit_dps(b, hc)
                rb = rbs.pop((b, hc))
                y16 = att['y16'].tile([P, T], f16, tag="y16",
                                      name=f"y16_{b}_{hc}")
                nc.vector.tensor_mul(y16[:], yps[:], rb[:])
                kp, sl8 = hc // 2, hc % 2
                yh_v = y8h[:, kp, b * TC8:(b + 1) * TC8, sl8, :]
                yl_v = y8l[:, kp, b * TC8:(b + 1) * TC8, sl8, :]
                nc.scalar.copy(yh_v, y16[:])
                with nc.allow_low_precision(
                        reason="fp8 hi/lo split; recon err ~0.1%, tol 2e-2"):
                    nc.gpsimd.tensor_sub(yl_v, y16[:], yh_v)

            def emit_proj(m, era='tail'):
                """out[tokens m*128:(m+1)*128, :] = y.T @ wproj (partial),
                3-term compensated fp8 DoubleRow."""
                psA = att['psA']
                def _cp_scalar(dst, srcv):
                    nc.scalar.copy(dst, srcv)

                def _cp_vector(dst, srcv):
                    nc.vector.tensor_copy(dst, srcv)

                engs = ((_cp_scalar, _cp_vector) if era == 'plan'
                        else (_cp_scalar, _cp_vector))
                ob = ob_pool.tile([P, NE], f16, tag="ob", name=f"ob{m}")
                for n in range(NE // 512):
                    opsum = psA.tile([P, 512], f32, tag="acc", bufs=6,
                                     name=f"ops{m}_{n}")
                    for kp in range(2):
                        for t, (yt, wv) in enumerate(
                                ((y8h, 0), (y8l, 0), (y8h, 1))):
                            nc.tensor.matmul(
                                opsum[:], yt[:, kp, m], wp8[:, kp, wv, n],
                                start=(kp == 0 and t == 0),
                                stop=(kp == 1 and t == 2), perf_mode=DR)
                    if 'tail_alt' in FLAGS and m == NT // P - 1:
                        if n % 2:
                            nc.vector.tensor_copy(
                                ob[:, n * 512:(n + 1) * 512], opsum[:])
                        else:
                            nc.scalar.copy(
                                ob[:, n * 512:(n + 1) * 512], opsum[:])
                    else:
                        engs[n % len(engs)](
                            ob[:, n * 512:(n + 1) * 512], opsum[:])
                    if 'tail_split' in FLAGS and m == NT // P - 1:
                        if n % 2 == 1:
                            c0, c1 = (n - 1) * 512, (n + 1) * 512
                            nc.sync.dma_start(
                                out_d[m * P:(m + 1) * P, c0:c1], ob[:, c0:c1])
                    elif 'quarter_dma' in FLAGS:
                        if n % 2 == 1:
                            c0, c1 = (n - 1) * 512, (n + 1) * 512
                            nc.sync.dma_start(
                                out_d[m * P:(m + 1) * P, c0:c1], ob[:, c0:c1])
                    elif 'half_dma' in FLAGS and n % 4 == 3:
                        c0, c1 = (n - 3) * 512, (n + 1) * 512
                        nc.sync.dma_start(
                            out_d[m * P:(m + 1) * P, c0:c1], ob[:, c0:c1])
                    elif n == 3:
                        nc.sync.dma_start(
                            out_d[m * P:(m + 1) * P, 0:2048], ob[:, 0:2048])
                    elif n == 7:
                        nc.sync.dma_start(
                            out_d[m * P:(m + 1) * P, 2048:NE], ob[:, 2048:NE])

            # ============ phase 1+2: qkv projection + rope, per batch ========
            with ExitStack() as sA:
                qkv_pool = sA.enter_context(tc.tile_pool(name="qkv", bufs=1))
                qkv = qkv_pool.tile([P, MC, NT], f16)
                wq_pool = sA.enter_context(tc.tile_pool(name="wq", bufs=1))
                wq8 = wq_pool.tile([P, KP, 2, MC, 2, P], f8)
                xs_pool = sA.enter_context(tc.tile_pool(name="xs", bufs=4))
                rp = sA.enter_context(tc.tile_pool(name="rope", bufs=2))

                def rope_span(b, tok, w):
                    h = HS // 2
                    ccb, ssb = cc[:, tok], ss[:, tok]
                    # half-spans get their own tags: mixed tile sizes inside
                    # one rotation tag alias SBUF and corrupt data on HW
                    sfx = "" if w == T else "H"
                    for hc in [QPK] + list(range(QPK)):
                        src_ = qkv[:, hc, tok]
                        rot = rp.tile([P, w], f16, tag="rot" + sfx,
                                      name=f"rot{b}_{hc}_{tok.start}")
                        nc.sync.dma_start(rot[0:h, :], src_[h:P, :])
                        nc.sync.dma_start(rot[h:P, :], src_[0:h, :])
                        t1 = rp.tile([P, w], f16, tag="t1" + sfx,
                                     name=f"t1_{b}_{hc}_{tok.start}")
                        t2 = rp.tile([P, w], f16, tag="t2" + sfx,
                                     name=f"t2_{b}_{hc}_{tok.start}")
                        nc.vector.tensor_mul(t1[:], src_, ccb)
                        nc.vector.tensor_mul(t2[:], rot[:], ssb)
                        dst = (q16[:, hc, tok] if hc < QPK
                               else k16[:, tok])
                        with nc.allow_low_precision(
                                reason="fp16 rope; |q|,|k|~1, tol 2e-2"):
                            nc.vector.tensor_add(dst, t1[:], t2[:])

                def rope_batch(b):
                    rope_span(b, slice(b * T, (b + 1) * T), T)

                def vt_batch(b, pool, tag, bufs, shape, cs=None):
                    for c in (range(TC8) if cs is None else cs):
                        # PE transpose (avoids XBAR DMA-transpose, which
                        # races concurrent DMA copies on this stack)
                        vt_ps = pool.tile(shape, f16, tag=tag, bufs=bufs,
                                          name=f"vt{b}_{c}")
                        nc.tensor.transpose(
                            vt_ps[:, 0:P],
                            qkv[:, QPK + 1, b * T + c * P: b * T + (c + 1) * P],
                            ident16)
                        nc.any.tensor_copy(
                            vtm[:, b * TC8 + c, :], vt_ps[:, 0:P])

                def consts_dma():
                    # one fused DMA for cos/sin tables, one for mask/ones/ident
                    nc.sync.dma_start(css[:], css_d[:])
                    nc.sync.dma_start(misc[:], misc_d[:])

                # resident x8 hi/lo tile sets shared by both batches;
                # hi+lo fused into one DMA to halve HWDGE desc-gen slots
                def load_xt(n, kp):
                    xt = xs_pool.tile([P, 2, 2, 512], f8, tag="xt",
                                      bufs=16, name=f"xt{n}_{kp}")
                    nc.sync.dma_start(xt[:], x8_d[n, kp])
                    return xt[:, 0], xt[:, 1]

                def qkv_mms(psums, ms, kp, xh, xl, first, last,
                            term_major=False):
                    """3-term compensated fp8 DR matmuls for one kp pair."""
                    terms = ((0, xh), (1, xh), (0, xl))
                    if term_major:
                        for t, (wv, xt) in enumerate(terms):
                            for m in ms:
                                nc.tensor.matmul(
                                    psums[m][:], wq8[:, kp, wv, m], xt,
                                    start=(first and t == 0),
                                    stop=(last and t == 2), perf_mode=DR)
                    else:
                        for m in ms:
                            for t, (wv, xt) in enumerate(terms):
                                nc.tensor.matmul(
                                    psums[m][:], wq8[:, kp, wv, m], xt,
                                    start=(first and t == 0),
                                    stop=(last and t == 2), perf_mode=DR)

                # ---- batch 0: flat 6-psum sweeps, own PSUM pool ----
                with ExitStack() as sP0:
                    ps1a = sP0.enter_context(
                        tc.tile_pool(name="ps1a", bufs=6, space="PSUM"))
                    for n in (0, 1):
                        psums = [ps1a.tile([P, 512], f32, tag="ps1",
                                           name=f"ps1_{n}_{m_}")
                                 for m_ in range(MC)]
                        for kp in range(KP):
                            if n == 0 and kp == 0:
                                # split the very first loads so PE starts on
                                # the hi-term sweep while lo is in flight
                                nc.sync.dma_start(wq8[:, 0, 0], wq8_d[0, :, 0])
                                xh, xl = load_xt(n, kp)
                                nc.sync.dma_start(wq8[:, 0, 1], wq8_d[0, :, 1])
                                qkv_mms(psums, range(MC), kp, xh, xl,
                                        True, False, term_major=True)
                                continue
                            if n == 0:
                                nc.sync.dma_start(wq8[:, kp], wq8_d[kp])
                            xh, xl = load_xt(n, kp)
                            qkv_mms(psums, range(MC), kp, xh, xl,
                                    kp == 0, kp == KP - 1)
                        for m in range(MC):
                            if 'altcopy' in FLAGS and m % 2 == 0:
                                nc.scalar.copy(
                                    qkv[:, m, n * 512:(n + 1) * 512],
                                    psums[m][:])
                            else:
                                nc.vector.tensor_copy(
                                    qkv[:, m, n * 512:(n + 1) * 512],
                                    psums[m][:])
                        if n == 0:
                            # MUST precede the first vt transpose: a
                            # reader emitted before its producer DMA
                            # gets no dependency and reads uninitialized
                            # SBUF (ident16 is vt's identity operand)
                            consts_dma()
                        if 'vt_split' in FLAGS:
                            vt_batch(0, ps1a, "vt", 2, [P, P],
                                     cs=range(n * 4, n * 4 + 4))
                    rope_batch(0)
                    if 'vt_split' not in FLAGS:
                        vt_batch(0, ps1a, "vt", 2, [P, P])

                # attention pools come alive before batch 1 so batch-0
                # scores/exp interleave into batch-1's qkv stream
                make_att_pools()

                # ---- batch 1: three 2-psum m-groups per n-chunk ----
                groups = ((0, 1), (2, 3), (4, 5))
                with ExitStack() as sP1:
                    ps1b = sP1.enter_context(
                        tc.tile_pool(name="ps1b", bufs=2, space="PSUM"))
                    for n in (2, 3):
                        xts = {}
                        for g, ms in enumerate(groups):
                            psums = {m_: ps1b.tile(
                                [P, 512], f32, tag="ps1",
                                name=f"ps1_{n}_{g}_{m_}") for m_ in ms}
                            for kp in range(KP):
                                if g == 0:
                                    xts[kp] = load_xt(n, kp)
                                qkv_mms(psums, ms, kp, *xts[kp],
                                        kp == 0, kp == KP - 1)
                            for m in ms:
                                nc.vector.tensor_copy(
                                    qkv[:, m, n * 512:(n + 1) * 512],
                                    psums[m][:])
                            slot = (n - 2) * 3 + g
                            if slot < QPK:
                                emit_scores(0, slot)
                    rope_batch(1)
                    vt_batch(1, att['psA'], "acc", 6, [P, 1024])

            # yps PSUM pool only comes alive after ps1b frees its banks
            att['psY'] = sR.enter_context(
                tc.tile_pool(name="psY", bufs=1, space="PSUM"))
            y8h, y8l, wp8 = alloc_yw_pools()
            ob_pool = sL.enter_context(tc.tile_pool(name="ob", bufs=2))
            for kp in range(2):
                nc.sync.dma_start(wp8[:, kp], wp8_d[kp])

            # ===== batch 0 attention (pv) / batch 1 scores interleave =====
            for i in range(QPK):
                emit_pv(0, i)
                emit_scores(1, i)

            # ===== batch 1 attention interleaved with batch-0 proj: proj
            # matmuls keep PE busy while ACT runs exp for the next head =====
            plan = [('pt', 0), ('pj', 0), ('pt', 1), ('pv', 0),
                    ('pj', 1), ('pt', 2), ('pj', 2), ('pv', 1),
                    ('pj', 3), ('pt', 3), ('pj', 4), ('pv', 2),
                    ('pj', 5), ('pj', 6), ('pv', 3), ('pj', 7)]
            for op, i in plan:
                if op == 'pt':
                    if 'late_part' in FLAGS:
                        emit_partial(1, i)
                elif op == 'pv':
                    emit_pv(1, i)
                else:
                    emit_proj(i, era='plan')
            for m in range(8, NT // P):
                emit_proj(m)
        finally:
            sR.close()
            sL.close()

    if split_waits:
        _split_waits(nc, mybir)
    return nc


def _q8(v):
    import ml_dtypes
    return np.ascontiguousarray(v).astype(ml_dtypes.float8_e4m3)


def _split8(v):
    """2-level e4m3 decomposition: v ~= hi + lo."""
    hi = _q8(v)
    lo = _q8(v - hi.astype(np.float32))
    return hi, lo


def _host_prep(x, cos, sin, W_attn, W_proj):
    xT = np.ascontiguousarray(x.reshape(NT, NE).T)          # [NE, NT] f32
    # x8[n, kp, p, which, i, c] = e4m3{,resid}(xT[(2kp+i)*P + p, n*512 + c])
    xr = xT.reshape(KP, 2, P, NNC, 512).transpose(3, 0, 2, 1, 4)
    x8h, x8l = _split8(xr)
    x8 = np.ascontiguousarray(np.stack([x8h, x8l], axis=3))
    cosT = np.tile(cos.T, (1, B)) / WSCALE
    sinT = np.tile(sin.T, (1, B)) / WSCALE
    cc = np.concatenate([cosT, cosT], axis=0)
    ss = np.concatenate([-sinT, sinT], axis=0)
    css = np.ascontiguousarray(
        np.stack([cc, ss], axis=1), dtype=np.float16)
    # scoresT layout [kv, q]: zero strictly-lower (kv > q) entries post-exp
    maskT = np.triu(np.ones((P, P), dtype=np.float32))
    misc = np.ascontiguousarray(np.stack(
        [maskT, np.ones((P, P), dtype=np.float32), np.eye(P)],
        axis=1), dtype=np.float16)
    common = {"x8": x8, "css": css, "misc": misc}
    in_maps = []
    for g in range(NCORES):
        m = dict(common)
        wq = W_attn[g * GW:(g + 1) * GW, :].T * WSCALE      # [NE, GW] f32
        # wq8[kp, p, which, m, i, j] = e4m3{,resid}(32*wq[(2kp+i)*P+p, m*128+j])
        wqr = wq.reshape(KP, 2, P, MC, P).transpose(0, 2, 3, 1, 4)
        m["wq8"] = np.ascontiguousarray(np.stack(_split8(wqr), axis=2))
        wp = W_proj[:, g * GQ:(g + 1) * GQ].T * WSCALE      # [GQ, NE] f32
        # wp8[kp, p, which, nn, i, c] = e4m3{,resid}(32*wp[(2kp+i)*P+p, nn*512+c])
        wpr = wp.reshape(2, 2, P, NE // 512, 512).transpose(0, 2, 3, 1, 4)
        m["wp8"] = np.ascontiguousarray(np.stack(_split8(wpr), axis=2))
        in_maps.append(m)
    return in_maps


LAST_EXEC_NS = None


def kernel(x, cos, sin, W_attn, W_proj, max_seq_length):
    global LAST_EXEC_NS
    from concourse.bass_utils import run_bass_kernel_spmd

    x = np.asarray(x, dtype=np.float32)
    cos = np.asarray(cos, dtype=np.float32)
    sin = np.asarray(sin, dtype=np.float32)
    W_attn = np.asarray(W_attn, dtype=np.float32)
    W_proj = np.asarray(W_proj, dtype=np.float32)

    if "nc" not in _CACHE:
        _CACHE["nc"] = _build_nc()
    nc = _CACHE["nc"]

    in_maps = _host_prep(x, cos, sin, W_attn, W_proj)
    res = run_bass_kernel_spmd(nc, in_maps, core_ids=list(range(NCORES)))
    LAST_EXEC_NS = res.exec_time_ns

    acc = res.results[0]["out"].astype(np.float32)
    for g in range(1, NCORES):
        acc = acc + res.results[g]["out"].astype(np.float32)
    return acc.reshape(B, T, NE) * (1.0 / (WSCALE * WSCALE))


# revision 18
# speedup vs baseline: 1.0242x; 1.0028x over previous
"""Trainium2 Bass kernel for CausalSelfAttention (GQA, RoPE, prefill).

Tensor-parallel over the 8 query groups: core g owns query heads
[4g, 4g+4) and kv head g.  Each core computes a partial output
(full-shape, fp16) that the host sums in fp32.

The two dense projections (qkv: x@W_attn slice, proj: y@W_proj slice)
run as 3-term error-compensated fp8 DoubleRow matmuls:
    x @ W  ~=  x_hi@W_hi + x_lo@W_hi + x_hi@W_lo
with x_hi = e4m3(x), x_lo = e4m3(x - x_hi) (same for W, pre-scaled by
32 on host so W's ~N(0, 1/4096) entries stay in e4m3 normal range).
DoubleRow packs two (128-contraction-plane, term) pairs per PE
instruction at 0.5 cycles per output column, so each term costs 1/4 of
an fp16 matmul and the compensated product runs at 0.75x fp16 time with
~0.2% error (vs ~4% for naive fp8).  Scale bookkeeping: q,k are
descaled by folding 1/32 into the host cos/sin tables; v stays 32x and
W_proj adds another 32x, so the host divides the summed output by 1024.

Attention stays fp16: scores KV-MAJOR (scoresT = kT.T @ qT, 6-deep PSUM
rotation), exp on ACT straight into the PV rhs layout, causal-diagonal
mask on GpSimd, softmax denominator via DVE block pre-reduce + one
ones-matmul (cross-partition reduce + broadcast) + DVE reciprocal.
emit_pv additionally splits y into fp8 hi/lo for the proj stage
(DVE mul, ACT quantize-copy, GpSimd residual-subtract).

Schedule skeleton (every engine stream is in-order, so EMISSION ORDER
is the schedule): batch-0 qkv runs flat 6-PSUM kp-sweeps; batch 1 runs
three 2-PSUM m-groups per n-chunk with batch-0 attention interleaved;
batch-0 proj chunks interleave into batch-1's attention as PE filler.
"""

import os
import numpy as np

FLAGS = set(os.environ.get(
    'KFLAGS',
    'half_dma,tail_split,tail_alt,altcopy,late_part,vt_split').split(','))

B, T, NE, NH, NQG, HS = 2, 1024, 4096, 32, 8, 128
QPK = NH // NQG          # 4 query heads per kv group
NT = B * T               # 2048 tokens
GW = (QPK + 2) * HS      # 768 qkv rows per group
GQ = QPK * HS            # 512 q cols per group
P = 128
NCORES = 8
KC = NE // P             # 32 contraction chunks for qkv proj
KP = KC // 2             # 16 DoubleRow plane-pairs
MC = GW // P             # 6 qkv feature chunks
TC8 = T // P             # 8 token chunks per batch
NNC = NT // 512          # 4 token n-chunks
SCALE = 1.0 / float(np.sqrt(HS))
WSCALE = 32.0            # host pre-scale on W_attn / W_proj before e4m3

_CACHE = {}


def _split_waits(nc, mybir, max_waits=1):
    """walrus in this container rejects >1 sync-wait per instruction;
    hoist extras onto single-wait NoOps just before (equivalent since
    semaphores are monotonic and a sequencer executes in order)."""
    for fn in nc.m.functions:
        for blk in fn.blocks:
            new_list, changed = [], False
            for inst in blk.instructions:
                si = getattr(inst, "sync_info", None)
                if si is not None and len(si.on_wait) > max_waits:
                    waits = list(si.on_wait)
                    for i, w in enumerate(waits[:-max_waits]):
                        nop = mybir.InstNoOp(
                            name=f"{inst.name}-wsplit-{i}", ins=[], outs=[],
                            engine=inst.engine)
                        nop.sync_info = mybir.SyncInfo(on_wait=[w], on_update=[])
                        new_list.append(nop)
                    inst.sync_info = mybir.SyncInfo(
                        on_wait=waits[-max_waits:], on_update=list(si.on_update))
                    changed = True
                new_list.append(inst)
            if changed:
                blk.instructions = new_list


def _build_nc(reps=1, split_waits=True):
    import concourse.bass as bass
    import concourse.mybir as mybir
    import concourse.tile as tile
    from contextlib import ExitStack

    f32 = mybir.dt.float32
    f16 = mybir.dt.float16
    f8 = mybir.dt.float8e4
    DR = mybir.MatmulPerfMode.DoubleRow

    nc = bass.Bass()
    # fp8 pair-packed inputs, hi/lo fused per DMA (see _host_prep)
    x8_d = nc.dram_tensor("x8", [NNC, KP, P, 2, 2, 512], f8,
                          kind="ExternalInput")
    wq8_d = nc.dram_tensor("wq8", [KP, P, 2, MC, 2, P], f8,
                           kind="ExternalInput")
    wp8_d = nc.dram_tensor("wp8", [2, P, 2, NE // 512, 2, 512], f8,
                           kind="ExternalInput")
    css_d = nc.dram_tensor("css", [P, 2, NT], f16, kind="ExternalInput")
    misc_d = nc.dram_tensor("misc", [P, 3, P], f16, kind="ExternalInput")
    out_d = nc.dram_tensor("out", [NT, NE], f16, kind="ExternalOutput")

    # column offset of kv-chunk c's block inside the expT tile
    offs, acc = [], 0
    for c in range(TC8):
        offs.append(acc)
        acc += (TC8 - c) * P

    with tile.TileContext(nc) as tc:
      for _rep in range(reps):
        sL = ExitStack()   # left-side long-lived pools (y8, wp8, ob)
        sR = ExitStack()   # right-side pools (qk16, attention-era)
        try:
            # const: left
            const = sL.enter_context(tc.tile_pool(name="const", bufs=1))
            css = const.tile([P, 2, NT], f16)
            cc, ss = css[:, 0], css[:, 1]
            misc = const.tile([P, 3, P], f16)
            maskT, ones2d, ident16 = misc[:, 0], misc[:, 1], misc[:, 2]

            def alloc_yw_pools():
                y_pool = sL.enter_context(tc.tile_pool(name="y", bufs=1))
                # (p, kp-pair, token-chunk, slot, col) — proj lhsT slices
                y8h = y_pool.tile([P, 2, NT // P, 2, P], f8)
                y8l = y_pool.tile([P, 2, NT // P, 2, P], f8)
                wp_pool = sL.enter_context(tc.tile_pool(name="wp", bufs=1))
                wp8 = wp_pool.tile([P, 2, 2, NE // 512, 2, 512], f8)
                return y8h, y8l, wp8

            # qk16 on the right: lives through attention
            qk16 = sR.enter_context(tc.tile_pool(name="qk16", bufs=1, side="right"))
            q16 = qk16.tile([P, QPK, NT], f16)
            k16 = qk16.tile([P, NT], f16)
            vtm = qk16.tile([P, B * TC8, P], f16)

            att = {}

            def make_att_pools():
                att['expT'] = sR.enter_context(
                    tc.tile_pool(name="expT", bufs=4, side="right"))
                att['part'] = sR.enter_context(
                    tc.tile_pool(name="part", bufs=2, side="right"))
                att['rb'] = sR.enter_context(
                    tc.tile_pool(name="rb", bufs=2, side="right"))
                att['y16'] = sR.enter_context(
                    tc.tile_pool(name="y16", bufs=2, side="right"))
                att['psA'] = sR.enter_context(
                    tc.tile_pool(name="psA", bufs=1, space="PSUM"))

            expTs = {}
            parts = {}
            rbs = {}

            def emit_partial(b, hc):
                """DVE pre-reduce of the kv-chunk blocks into partial[128,T]
                (softmax denominator before the cross-partition reduce).
                Emitted only once exp data is near-ready so the in-order DVE
                queue is not head-of-line blocked."""
                expT = expTs[(b, hc)]
                eng = nc.vector
                part = att['part'].tile([P, T], f16, tag="part",
                                        name=f"part{b}_{hc}")
                parts[(b, hc)] = part
                eng.tensor_copy(part[:], expT[:, offs[0]:offs[0] + T])
                with nc.allow_low_precision(
                        reason="fp16 partial rowsums; d<=~3e3, tol 2e-2"):
                    for c in range(1, TC8):
                        w = T - c * P
                        eng.tensor_add(
                            part[:, c * P:T], part[:, c * P:T],
                            expT[:, offs[c]:offs[c] + w])

            def emit_dps(b, hc):
                """softmax denominator: ones-matmul does the cross-partition
                reduce AND the 128-way broadcast; DVE reciprocal to SBUF."""
                psA = att['psA']
                part = parts.pop((b, hc))
                rb = att['rb'].tile([P, T], f16, tag="rb", name=f"rb{b}_{hc}")
                rbs[(b, hc)] = rb
                for (q0, q1) in ((0, 512), (512, T)):
                    dps = psA.tile([P, 512], f32, tag="acc", bufs=6,
                                   name=f"dps{b}_{hc}_{q0}")
                    nc.tensor.matmul(dps[:], ones2d, part[:, q0:q1],
                                     start=True, stop=True)
                    with nc.allow_low_precision(
                            reason="fp16 1/d; d in [1,~3e3], tol 2e-2"):
                        nc.vector.reciprocal(rb[:, q0:q1], dps[:])

            def emit_scores(b, hc):
                """scoresT = kT.T @ qT per kv chunk, exp on ACT, causal mask
                on GpSimd."""
                psA = att['psA']
                tok = slice(b * T, (b + 1) * T)
                qT_i = q16[:, hc, tok]
                expT = att['expT'].tile([P, acc], f16, tag="expT",
                                        name=f"expT{b}_{hc}")
                expTs[(b, hc)] = expT
                for c in range(TC8):
                    kT_c = k16[:, b * T + c * P: b * T + (c + 1) * P]
                    spans = [(c * P, 512)] if c < 4 else []
                    spans += [(max(512, c * P), T)]
                    for si, (q0, q1) in enumerate(spans):
                        sps = psA.tile([P, 512], f32, tag="acc", bufs=6,
                                       name=f"sps{b}_{hc}_{c}_{q0}")
                        w = q1 - q0
                        nc.tensor.matmul(sps[:, :w], kT_c,
                                         qT_i[:, q0:q1],
                                         start=True, stop=True)
                        eo = offs[c] + (q0 - c * P)
                        nc.scalar.activation(
                            expT[:, eo:eo + w], sps[:, :w],
                            mybir.ActivationFunctionType.Exp, scale=SCALE)
                    # zero the invalid (kv > q) half of the diagonal block
                    nc.gpsimd.tensor_mul(
                        expT[:, offs[c]:offs[c] + P],
                        expT[:, offs[c]:offs[c] + P], maskT)
                if not (b == 1 and 'late_part' in FLAGS):
                    emit_partial(b, hc)

            def emit_pv(b, hc):
                """y = probs @ v (unnormalized), denominator reduce+broadcast
                via ones-matmul, DVE reciprocal, normalizing multiply into a
                fp16 scratch, then fp8 hi/lo split for the proj stage."""
                psA = att['psA']
                if (b, hc) not in parts and (b, hc) not in rbs:
                    emit_partial(b, hc)
                expT = expTs.pop((b, hc))
                yps = att['psY'].tile([P, T], f32, tag="yps", bufs=1,
                                      name=f"yps{b}_{hc}")
                for (s0, s1) in ((0, 512), (512, T)):
                    cs = [c for c in range(TC8) if c * P < s1]
                    for c in cs:
                        q0 = max(s0, c * P)
                        sl = slice(offs[c] + (q0 - c * P),
                                   offs[c] + (s1 - c * P))
                        nc.tensor.matmul(
                            yps[:, q0:s1], vtm[:, b * TC8 + c, :],
                            expT[:, sl], start=(c == 0), stop=(c == cs[-1]))
                if (b, hc) not in rbs:
                    emit_dps(b, hc)
                rb = rbs.pop((b, hc))
                y16 = att['y16'].tile([P, T], f16, tag="y16",
                                      name=f"y16_{b}_{hc}")
                nc.vector.tensor_mul(y16[:], yps[:], rb[:])
                kp, sl8 = hc // 2, hc % 2
                yh_v = y8h[:, kp, b * TC8:(b + 1) * TC8, sl8, :]
                yl_v = y8l[:, kp, b * TC8:(b + 1) * TC8, sl8, :]
                nc.scalar.copy(yh_v, y16[:])
                with nc.allow_low_precision(
                        reason="fp8 hi/lo split; recon err ~0.1%, tol 2e-2"):
                    nc.gpsimd.tensor_sub(yl_v, y16[:], yh_v)

            def emit_proj(m, era='tail'):
                """out[tokens m*128:(m+1)*128, :] = y.T @ wproj (partial),
                3-term compensated fp8 DoubleRow."""
                psA = att['psA']
                def _cp_scalar(dst, srcv):
                    nc.scalar.copy(dst, srcv)

                def _cp_vector(dst, srcv):
                    nc.vector.tensor_copy(dst, srcv)

                engs = ((_cp_scalar, _cp_vector) if era == 'plan'
                        else (_cp_scalar, _cp_vector))
                ob = ob_pool.tile([P, NE], f16, tag="ob", name=f"ob{m}")
                for n in range(NE // 512):
                    opsum = psA.tile([P, 512], f32, tag="acc", bufs=6,
                                     name=f"ops{m}_{n}")
                    for kp in range(2):
                        for t, (yt, wv) in enumerate(
                                ((y8h, 0), (y8l, 0), (y8h, 1))):
                            nc.tensor.matmul(
                                opsum[:], yt[:, kp, m], wp8[:, kp, wv, n],
                                start=(kp == 0 and t == 0),
                                stop=(kp == 1 and t == 2), perf_mode=DR)
                    if 'tail_alt' in FLAGS and m == NT // P - 1:
                        if n % 2:
                            nc.vector.tensor_copy(
                                ob[:, n * 512:(n + 1) * 512], opsum[:])
                        else:
                            nc.scalar.copy(
                                ob[:, n * 512:(n + 1) * 512], opsum[:])
                    else:
                        engs[n % len(engs)](
                            ob[:, n * 512:(n + 1) * 512], opsum[:])
                    if 'tail_split' in FLAGS and m == NT // P - 1:
                        if n % 2 == 1:
                            c0, c1 = (n - 1) * 512, (n + 1) * 512
                            nc.sync.dma_start(
                                out_d[m * P:(m + 1) * P, c0:c1], ob[:, c0:c1])
                    elif 'quarter_dma' in FLAGS:
                        if n % 2 == 1:
                            c0, c1 = (n - 1) * 512, (n + 1) * 512
                            nc.sync.dma_start(
                                out_d[m * P:(m + 1) * P, c0:c1], ob[:, c0:c1])
                    elif 'half_dma' in FLAGS and n % 4 == 3:
                        c0, c1 = (n - 3) * 512, (n + 1) * 512
                        nc.sync.dma_start(
                            out_d[m * P:(m + 1) * P, c0:c1], ob[:, c0:c1])
                    elif n == 3:
                        nc.sync.dma_start(
                            out_d[m * P:(m + 1) * P, 0:2048], ob[:, 0:2048])
                    elif n == 7:
                        nc.sync.dma_start(
                            out_d[m * P:(m + 1) * P, 2048:NE], ob[:, 2048:NE])

            # ============ phase 1+2: qkv projection + rope, per batch ========
            with ExitStack() as sA:
                qkv_pool = sA.enter_context(tc.tile_pool(name="qkv", bufs=1))
                qkv = qkv_pool.tile([P, MC, NT], f16)
                wq_pool = sA.enter_context(tc.tile_pool(name="wq", bufs=1))
                wq8 = wq_pool.tile([P, KP, 2, MC, 2, P], f8)
                xs_pool = sA.enter_context(tc.tile_pool(name="xs", bufs=4))
                rp = sA.enter_context(tc.tile_pool(name="rope", bufs=2))

                def rope_span(b, tok, w):
                    h = HS // 2
                    ccb, ssb = cc[:, tok], ss[:, tok]
                    # half-spans get their own tags: mixed tile sizes inside
                    # one rotation tag alias SBUF and corrupt data on HW
                    sfx = "" if w == T else "H"
                    for hc in [QPK] + list(range(QPK)):
                        src_ = qkv[:, hc, tok]
                        rot = rp.tile([P, w], f16, tag="rot" + sfx,
                                      name=f"rot{b}_{hc}_{tok.start}")
                        nc.sync.dma_start(rot[0:h, :], src_[h:P, :])
                        nc.sync.dma_start(rot[h:P, :], src_[0:h, :])
                        t1 = rp.tile([P, w], f16, tag="t1" + sfx,
                                     name=f"t1_{b}_{hc}_{tok.start}")
                        t2 = rp.tile([P, w], f16, tag="t2" + sfx,
                                     name=f"t2_{b}_{hc}_{tok.start}")
                        nc.vector.tensor_mul(t1[:], src_, ccb)
                        nc.vector.tensor_mul(t2[:], rot[:], ssb)
                        dst = (q16[:, hc, tok] if hc < QPK
                               else k16[:, tok])
                        with nc.allow_low_precision(
                                reason="fp16 rope; |q|,|k|~1, tol 2e-2"):
                            nc.vector.tensor_add(dst, t1[:], t2[:])

                def rope_batch(b):
                    rope_span(b, slice(b * T, (b + 1) * T), T)

                def rope_nhalf(n):
                    rope_span(n // 2, slice(n * 512, (n + 1) * 512), 512)

                def vt_batch(b, pool, tag, bufs, shape, cs=None):
                    for c in (range(TC8) if cs is None else cs):
                        # PE transpose (avoids XBAR DMA-transpose, which
                        # races concurrent DMA copies on this stack)
                        vt_ps = pool.tile(shape, f16, tag=tag, bufs=bufs,
                                          name=f"vt{b}_{c}")
                        nc.tensor.transpose(
                            vt_ps[:, 0:P],
                            qkv[:, QPK + 1, b * T + c * P: b * T + (c + 1) * P],
                            ident16)
                        nc.any.tensor_copy(
                            vtm[:, b * TC8 + c, :], vt_ps[:, 0:P])

                def consts_dma():
                    # one fused DMA for cos/sin tables, one for mask/ones/ident
                    nc.sync.dma_start(css[:], css_d[:])
                    nc.sync.dma_start(misc[:], misc_d[:])

                # resident x8 hi/lo tile sets shared by both batches;
                # hi+lo fused into one DMA to halve HWDGE desc-gen slots
                def load_xt(n, kp):
                    xt = xs_pool.tile([P, 2, 2, 512], f8, tag="xt",
                                      bufs=16, name=f"xt{n}_{kp}")
                    nc.sync.dma_start(xt[:], x8_d[n, kp])
                    return xt[:, 0], xt[:, 1]

                def qkv_mms(psums, ms, kp, xh, xl, first, last,
                            term_major=False):
                    """3-term compensated fp8 DR matmuls for one kp pair."""
                    terms = ((0, xh), (1, xh), (0, xl))
                    if term_major:
                        for t, (wv, xt) in enumerate(terms):
                            for m in ms:
                                nc.tensor.matmul(
                                    psums[m][:], wq8[:, kp, wv, m], xt,
                                    start=(first and t == 0),
                                    stop=(last and t == 2), perf_mode=DR)
                    else:
                        for m in ms:
                            for t, (wv, xt) in enumerate(terms):
                                nc.tensor.matmul(
                                    psums[m][:], wq8[:, kp, wv, m], xt,
                                    start=(first and t == 0),
                                    stop=(last and t == 2), perf_mode=DR)

                # ---- batch 0: flat 6-psum sweeps, own PSUM pool ----
                with ExitStack() as sP0:
                    ps1a = sP0.enter_context(
                        tc.tile_pool(name="ps1a", bufs=6, space="PSUM"))
                    for n in (0, 1):
                        psums = [ps1a.tile([P, 512], f32, tag="ps1",
                                           name=f"ps1_{n}_{m_}")
                                 for m_ in range(MC)]
                        for kp in range(KP):
                            if n == 0 and kp == 0:
                                # split the very first loads so PE starts on
                                # the hi-term sweep while lo is in flight
                                nc.sync.dma_start(wq8[:, 0, 0, 0],
                                                  wq8_d[0, :, 0, 0])
                                nc.sync.dma_start(wq8[:, 0, 0, 1:],
                                                  wq8_d[0, :, 0, 1:])
                                xh, xl = load_xt(n, kp)
                                nc.sync.dma_start(wq8[:, 0, 1], wq8_d[0, :, 1])
                                qkv_mms(psums, range(MC), kp, xh, xl,
                                        True, False, term_major=True)
                                continue
                            if n == 0:
                                nc.sync.dma_start(wq8[:, kp], wq8_d[kp])
                            xh, xl = load_xt(n, kp)
                            qkv_mms(psums, range(MC), kp, xh, xl,
                                    kp == 0, kp == KP - 1)
                        for m in range(MC):
                            if 'altcopy' in FLAGS and m % 2 == 0:
                                nc.scalar.copy(
                                    qkv[:, m, n * 512:(n + 1) * 512],
                                    psums[m][:])
                            else:
                                nc.vector.tensor_copy(
                                    qkv[:, m, n * 512:(n + 1) * 512],
                                    psums[m][:])
                        if n == 0:
                            # MUST precede the first vt transpose: a
                            # reader emitted before its producer DMA
                            # gets no dependency and reads uninitialized
                            # SBUF (ident16 is vt's identity operand)
                            consts_dma()
                        if 'vt_split' in FLAGS:
                            vt_batch(0, ps1a, "vt", 2, [P, P],
                                     cs=range(n * 4, n * 4 + 4))
                    rope_batch(0)
                    if 'vt_split' not in FLAGS:
                        vt_batch(0, ps1a, "vt", 2, [P, P])

                # attention pools come alive before batch 1 so batch-0
                # scores/exp interleave into batch-1's qkv stream
                make_att_pools()

                # ---- batch 1: three 2-psum m-groups per n-chunk ----
                groups = ((0, 1), (2, 3), (4, 5))
                with ExitStack() as sP1:
                    ps1b = sP1.enter_context(
                        tc.tile_pool(name="ps1b", bufs=2, space="PSUM"))
                    for n in (2, 3):
                        xts = {}
                        for g, ms in enumerate(groups):
                            psums = {m_: ps1b.tile(
                                [P, 512], f32, tag="ps1",
                                name=f"ps1_{n}_{g}_{m_}") for m_ in ms}
                            for kp in range(KP):
                                if g == 0:
                                    xts[kp] = load_xt(n, kp)
                                qkv_mms(psums, ms, kp, *xts[kp],
                                        kp == 0, kp == KP - 1)
                            for m in ms:
                                nc.vector.tensor_copy(
                                    qkv[:, m, n * 512:(n + 1) * 512],
                                    psums[m][:])
                            slot = (n - 2) * 3 + g
                            if slot < QPK:
                                emit_scores(0, slot)
                        rope_nhalf(n)
                    vt_batch(1, att['psA'], "acc", 6, [P, 1024])

            # yps PSUM pool only comes alive after ps1b frees its banks
            att['psY'] = sR.enter_context(
                tc.tile_pool(name="psY", bufs=1, space="PSUM"))
            y8h, y8l, wp8 = alloc_yw_pools()
            ob_pool = sL.enter_context(tc.tile_pool(name="ob", bufs=2))
            for kp in range(2):
                nc.sync.dma_start(wp8[:, kp], wp8_d[kp])

            # ===== batch 0 attention (pv) / batch 1 scores interleave =====
            for i in range(QPK):
                emit_pv(0, i)
                emit_scores(1, i)

            # ===== batch 1 attention interleaved with batch-0 proj: proj
            # matmuls keep PE busy while ACT runs exp for the next head =====
            plan = [('pt', 0), ('pj', 0), ('pt', 1), ('pv', 0),
                    ('pj', 1), ('pt', 2), ('pj', 2), ('pv', 1),
                    ('pj', 3), ('pt', 3), ('pj', 4), ('pv', 2),
                    ('pj', 5), ('pj', 6), ('pv', 3), ('pj', 7)]
            for op, i in plan:
                if op == 'pt':
                    if 'late_part' in FLAGS:
                        emit_partial(1, i)
                elif op == 'pv':
                    emit_pv(1, i)
                else:
                    emit_proj(i, era='plan')
            for m in range(8, NT // P):
                emit_proj(m)
        finally:
            sR.close()
            sL.close()

    if split_waits:
        _split_waits(nc, mybir)
    return nc


def _q8(v):
    import ml_dtypes
    return np.ascontiguousarray(v).astype(ml_dtypes.float8_e4m3)


def _split8(v):
    """2-level e4m3 decomposition: v ~= hi + lo."""
    hi = _q8(v)
    lo = _q8(v - hi.astype(np.float32))
    return hi, lo


def _host_prep(x, cos, sin, W_attn, W_proj):
    xT = np.ascontiguousarray(x.reshape(NT, NE).T)          # [NE, NT] f32
    # x8[n, kp, p, which, i, c] = e4m3{,resid}(xT[(2kp+i)*P + p, n*512 + c])
    xr = xT.reshape(KP, 2, P, NNC, 512).transpose(3, 0, 2, 1, 4)
    x8h, x8l = _split8(xr)
    x8 = np.ascontiguousarray(np.stack([x8h, x8l], axis=3))
    cosT = np.tile(cos.T, (1, B)) / WSCALE
    sinT = np.tile(sin.T, (1, B)) / WSCALE
    cc = np.concatenate([cosT, cosT], axis=0)
    ss = np.concatenate([-sinT, sinT], axis=0)
    css = np.ascontiguousarray(
        np.stack([cc, ss], axis=1), dtype=np.float16)
    # scoresT layout [kv, q]: zero strictly-lower (kv > q) entries post-exp
    maskT = np.triu(np.ones((P, P), dtype=np.float32))
    misc = np.ascontiguousarray(np.stack(
        [maskT, np.ones((P, P), dtype=np.float32), np.eye(P)],
        axis=1), dtype=np.float16)
    common = {"x8": x8, "css": css, "misc": misc}
    in_maps = []
    for g in range(NCORES):
        m = dict(common)
        wq = W_attn[g * GW:(g + 1) * GW, :].T * WSCALE      # [NE, GW] f32
        # wq8[kp, p, which, m, i, j] = e4m3{,resid}(32*wq[(2kp+i)*P+p, m*128+j])
        wqr = wq.reshape(KP, 2, P, MC, P).transpose(0, 2, 3, 1, 4)
        m["wq8"] = np.ascontiguousarray(np.stack(_split8(wqr), axis=2))
        wp = W_proj[:, g * GQ:(g + 1) * GQ].T * WSCALE      # [GQ, NE] f32
        # wp8[kp, p, which, nn, i, c] = e4m3{,resid}(32*wp[(2kp+i)*P+p, nn*512+c])
        wpr = wp.reshape(2, 2, P, NE // 512, 512).transpose(0, 2, 3, 1, 4)
        m["wp8"] = np.ascontiguousarray(np.stack(_split8(wpr), axis=2))
        in_maps.append(m)
    return in_maps


LAST_EXEC_NS = None


def kernel(x, cos, sin, W_attn, W_proj, max_seq_length):
    global LAST_EXEC_NS
    from concourse.bass_utils import run_bass_kernel_spmd

    x = np.asarray(x, dtype=np.float32)
    cos = np.asarray(cos, dtype=np.float32)
    sin = np.asarray(sin, dtype=np.float32)
    W_attn = np.asarray(W_attn, dtype=np.float32)
    W_proj = np.asarray(W_proj, dtype=np.float32)

    if "nc" not in _CACHE:
        _CACHE["nc"] = _build_nc()
    nc = _CACHE["nc"]

    in_maps = _host_prep(x, cos, sin, W_attn, W_proj)
    res = run_bass_kernel_spmd(nc, in_maps, core_ids=list(range(NCORES)))
    LAST_EXEC_NS = res.exec_time_ns

    acc = res.results[0]["out"].astype(np.float32)
    for g in range(1, NCORES):
        acc = acc + res.results[g]["out"].astype(np.float32)
    return acc.reshape(B, T, NE) * (1.0 / (WSCALE * WSCALE))
